# revision 12
# baseline (speedup 1.0000x reference)
"""Multi-head attention kernel for 8 Trainium2 NeuronCores.

Problem: embeddings [4, 2048, 1024], 16 heads x 64 dim, torch nn.Linear
convention (x @ W.T + b) for Q/K/V/O projections.

Sharding: batch (4) x head-halves (2) -> 8 cores. Core c handles batch
c//2, local heads (c%2)*8..(c%2)*8+8. Output projection is row-sharded;
host sums the two partial outputs per batch element and adds bo.

Per-core dataflow (feature dims on partitions, ScalarE exp is the
critical engine and is kept fed by interleaving projection work into
the attention instruction stream):
  xT [1024e, 2048t] bf16 (host pre-transposed + cast)
  QT/KT [(h,d)=512, t] via PE, bias added on DVE during PSUM evac.
  V [t, (h,d)] via PE, bias via K=1 ones x bv matmul.
  Per head-quad (4 heads = 2 pairs), per q-block of 512, per k-tile:
    scores_T[k,q] row-paired matmuls (2 heads share the PE array),
    exp on ScalarE (1/8 scale folded in, no max subtraction needed),
    U[(2x64),q] col-paired matmuls, sumexp via 4 col-tiled M=1
    ones-matmuls into one PSUM bank (partitions 0/32/64/96).
  normalize: recip(sumexp) -> gpsimd partition-broadcast -> DVE mult.
  yT[e_out, t] = woT.T @ attn_T accumulated over 4 pair-tiles.
Host: out[b] = (yT[2b] + yT[2b+1]).T + bo.
"""

import sys

sys.path.insert(0, "/opt/trn_rl_repo")

import numpy as np
import ml_dtypes

import concourse.bass as bass
import concourse.bacc as bacc
import concourse.mybir as mybir
import concourse.tile as tile
from concourse.bass_utils import run_bass_kernel_spmd

BF16 = mybir.dt.bfloat16
F32 = mybir.dt.float32
NPBF16 = ml_dtypes.bfloat16

B, S, E = 4, 2048, 1024
H_LOC = 8          # local heads per core
D = 64             # head dim
OL = H_LOC * D     # 512 local output dim
N_CORES = 8
QB = 512           # query block (free dim of scores_T)
NQB = S // QB      # 4
NKT = S // 128     # 16 key tiles
NET = E // 128     # 8 embed tiles


def build_program():
    from contextlib import ExitStack

    nc = bacc.Bacc("TRN2", debug=False, num_devices=N_CORES)

    xT = nc.dram_tensor("xT", [E, S], BF16, kind="ExternalInput")
    wqT = nc.dram_tensor("wqT", [E, OL], BF16, kind="ExternalInput")
    wkT = nc.dram_tensor("wkT", [E, OL], BF16, kind="ExternalInput")
    wvT = nc.dram_tensor("wvT", [E, OL], BF16, kind="ExternalInput")
    woT = nc.dram_tensor("woT", [OL, E], BF16, kind="ExternalInput")
    bqc = nc.dram_tensor("bqc", [128, 4], F32, kind="ExternalInput")
    bkc = nc.dram_tensor("bkc", [128, 4], F32, kind="ExternalInput")
    bv = nc.dram_tensor("bv", [1, OL], BF16, kind="ExternalInput")
    yT = nc.dram_tensor("yT", [E, S], F32, kind="ExternalOutput")

    with tile.TileContext(nc) as tc, ExitStack() as est:
        xt_p = est.enter_context(tc.tile_pool(name="xt", bufs=NET))
        wq_p = est.enter_context(tc.tile_pool(name="wq", bufs=NET))
        wk_p = est.enter_context(tc.tile_pool(name="wk", bufs=NET))
        wv_p = est.enter_context(tc.tile_pool(name="wv", bufs=NET))
        wo_p = est.enter_context(tc.tile_pool(name="wo", bufs=4))
        bias_p = est.enter_context(tc.tile_pool(name="bias", bufs=4))
        qt_p = est.enter_context(tc.tile_pool(name="qt", bufs=4))
        kt_p = est.enter_context(tc.tile_pool(name="kt", bufs=4))
        vb_p = est.enter_context(tc.tile_pool(name="vb", bufs=NKT))
        pj_p = est.enter_context(tc.tile_pool(name="pj", bufs=1, space="PSUM"))
        sc_p = est.enter_context(tc.tile_pool(name="sc", bufs=2, space="PSUM"))
        u_p = est.enter_context(tc.tile_pool(name="u", bufs=2, space="PSUM"))
        se_p = est.enter_context(tc.tile_pool(name="se", bufs=1, space="PSUM"))
        ex_p = est.enter_context(tc.tile_pool(name="ex", bufs=4))
        at_p = est.enter_context(tc.tile_pool(name="at", bufs=16))
        nrm_p = est.enter_context(tc.tile_pool(name="nrm", bufs=2))
        ys_p = est.enter_context(tc.tile_pool(name="ys", bufs=2))

        # ---- load inputs ----
        xts = []
        for e in range(NET):
            t = xt_p.tile([128, S], BF16, tag="xt", name="xt")
            nc.sync.dma_start(t[:], xT[e * 128:(e + 1) * 128, :])
            xts.append(t)
        wts = {}
        for name, dram, pool in (
            ("q", wqT, wq_p), ("k", wkT, wk_p), ("v", wvT, wv_p),
        ):
            lst = []
            for e in range(NET):
                t = pool.tile([128, OL], BF16, tag="w" + name, name="w" + name)
                nc.sync.dma_start(t[:], dram[e * 128:(e + 1) * 128, :])
                lst.append(t)
            wts[name] = lst
        wos = []
        for p in range(4):
            t = wo_p.tile([128, E], BF16, tag="wo", name="wo")
            nc.sync.dma_start(t[:], woT[p * 128:(p + 1) * 128, :])
            wos.append(t)
        bqs = bias_p.tile([128, 4], F32, tag="bqc")
        bks = bias_p.tile([128, 4], F32, tag="bkc")
        bvs = bias_p.tile([1, OL], BF16, tag="bv")
        ones = bias_p.tile([1, 128], BF16, tag="ones")
        onecol = bias_p.tile([128, 1], BF16, tag="onecol")
        nc.sync.dma_start(bqs[:], bqc[:])
        nc.sync.dma_start(bks[:], bkc[:])
        nc.sync.dma_start(bvs[:], bv[:])
        nc.vector.memset(ones[:], 1.0)
        nc.vector.memset(onecol[:], 1.0)

        qts = [qt_p.tile([128, S], BF16, tag="qt", name="qt")
               for _ in range(4)]
        kts = [kt_p.tile([128, S], BF16, tag="kt", name="kt")
               for _ in range(4)]
        vbs = [vb_p.tile([128, OL], BF16, tag="vb", name="vb")
               for _ in range(NKT)]
        atts = [[at_p.tile([128, QB], BF16, tag="at", name="at")
                 for _ in range(4)] for _ in range(NQB)]

        # ---- projection / outproj group emitters (PE fillers) ----
        def qk_group(i, j, which):
            """Q or K projection for o-tile i, t-block j (one PSUM group)."""
            w = wts[which]
            bias_t = bqs if which == "q" else bks
            dest = qts[i] if which == "q" else kts[i]
            acc = pj_p.tile([128, QB], F32, tag="pj", name="pj")
            for e in range(NET):
                nc.tensor.matmul(
                    acc[:],
                    w[e][:, i * 128:(i + 1) * 128],
                    xts[e][:, j * QB:(j + 1) * QB],
                    start=(e == 0), stop=(e == NET - 1),
                )
            nc.vector.tensor_scalar_add(
                dest[:, j * QB:(j + 1) * QB], acc[:], bias_t[:, i:i + 1])

        def v_group(ti):
            acc = pj_p.tile([128, OL], F32, tag="pj", name="pjv")
            nc.tensor.matmul(
                acc[:], ones[0:1, :], bvs[0:1, :], start=True, stop=False)
            for e in range(NET):
                nc.tensor.matmul(
                    acc[:],
                    xts[e][:, ti * 128:(ti + 1) * 128],
                    wts["v"][e][:],
                    start=False, stop=(e == NET - 1),
                )
            nc.vector.tensor_copy(vbs[ti][:], acc[:])

        def outproj_group(qb, eo):
            y = pj_p.tile([128, QB], F32, tag="pj", name="y")
            for p2 in range(4):
                nc.tensor.matmul(
                    y[:],
                    wos[p2][:, eo * 128:(eo + 1) * 128],
                    atts[qb][p2][:],
                    start=(p2 == 0), stop=(p2 == 3),
                )
            ysb = ys_p.tile([128, QB], F32, tag="ys", name="ys")
            nc.vector.tensor_copy(ysb[:], y[:])
            nc.sync.dma_start(
                yT[eo * 128:(eo + 1) * 128, qb * QB:(qb + 1) * QB], ysb[:])

        # ---- filler schedule ----
        def qg(i, j):
            return lambda: qk_group(i, j, "q")

        def kg(i, j):
            return lambda: qk_group(i, j, "k")

        def vg(t):
            return lambda: v_group(t)

        def og(qb, e):
            return lambda: outproj_group(qb, e)

        # K tiles span the whole sequence: K(pair, j) is needed from
        # qb0 kt=4j on. V(t) is needed at qb0 kt=t. Q(pair, j) at qb=j.
        prologue = [qg(0, 0), qg(1, 0), kg(0, 0), kg(1, 0),
                    vg(0), vg(1), vg(2)]

        def spread(thunks):
            "Distribute thunks evenly over the 16 k-tile slots."
            d = {}
            n = len(thunks)
            for idx, th in enumerate(thunks):
                d.setdefault(idx * NKT // n, []).append(th)
            return d

        fillers = {}  # (quad, qb) -> {kt: [thunks]}, emitted at top of kt
        # qb0 of quad0 has hard deadlines: V(t) before U at kt=t,
        # K(pair, j) before scores at kt=4j.
        fillers[(0, 0)] = {
            0: [vg(3), kg(0, 1)], 1: [vg(4), kg(1, 1)], 2: [vg(5), vg(6)],
            3: [vg(7), kg(0, 2)], 4: [vg(8), kg(1, 2)], 5: [vg(9), vg(10)],
            6: [vg(11), kg(0, 3)], 7: [vg(12), kg(1, 3)],
            8: [vg(13), vg(14)], 9: [vg(15)], 10: [qg(0, 1)],
            11: [qg(1, 1)]}
        fillers[(0, 1)] = spread([qg(0, 2), qg(1, 2), kg(2, 0), kg(2, 1),
                                  kg(3, 0), kg(3, 1)])
        fillers[(0, 2)] = spread([qg(0, 3), qg(1, 3), kg(2, 2), kg(2, 3),
                                  kg(3, 2), kg(3, 3)])
        fillers[(0, 3)] = spread([qg(2, 0), qg(3, 0)])
        fillers[(1, 0)] = spread([qg(2, 1), qg(3, 1)])
        fillers[(1, 1)] = spread([qg(2, 2), qg(3, 2)]
                                 + [og(0, e) for e in range(NET)])
        fillers[(1, 2)] = spread([qg(2, 3), qg(3, 3)]
                                 + [og(1, e) for e in range(NET)])
        fillers[(1, 3)] = spread([og(2, e) for e in range(NET)])

        for th in prologue:
            th()

        # ---- attention: head-quads x q-blocks x k-tiles ----
        for quad in range(2):
            pA, pB = 2 * quad, 2 * quad + 1
            for qb in range(NQB):
                fl = fillers.get((quad, qb), {})
                uA = u_p.tile([128, QB], F32, tag="u", name="uA")
                uB = u_p.tile([128, QB], F32, tag="u", name="uB")
                seb = se_p.tile([128, QB], F32, tag="se", name="seb")
                for kt in range(NKT):
                    for th in fl.get(kt, []):
                        th()
                    ets = []
                    for pair in (pA, pB):
                        sc = sc_p.tile([128, 2 * QB], F32, tag="sc",
                                       name="sc")
                        nc.tensor.matmul(
                            sc[:, 0:QB],
                            kts[pair][0:64, kt * 128:(kt + 1) * 128],
                            qts[pair][0:64, qb * QB:(qb + 1) * QB],
                            start=True, stop=True, tile_position=(0, 0),
                        )
                        nc.tensor.matmul(
                            sc[:, QB:2 * QB],
                            kts[pair][64:128, kt * 128:(kt + 1) * 128],
                            qts[pair][64:128, qb * QB:(qb + 1) * QB],
                            start=True, stop=True, tile_position=(64, 0),
                        )
                        et = ex_p.tile([128, 2 * QB], BF16, tag="ex",
                                       name="ex")
                        nc.scalar.activation(
                            et[:], sc[:],
                            mybir.ActivationFunctionType.Exp, scale=0.125)
                        ets.append(et)
                    for pi, u in enumerate((uA, uB)):
                        pair = (pA, pB)[pi]
                        for sub in range(2):
                            hcol = (pair * 2 + sub) * D
                            nc.tensor.matmul(
                                u[sub * 64:(sub + 1) * 64, :],
                                vbs[kt][:, hcol:hcol + D],
                                ets[pi][:, sub * QB:(sub + 1) * QB],
                                start=(kt == 0), stop=(kt == NKT - 1),
                                tile_position=(0, sub * 64),
                                skip_group_check=True,
                            )
                    for g in range(4):
                        nc.tensor.matmul(
                            seb[g * 32:g * 32 + 1, :],
                            onecol[:],
                            ets[g // 2][:, (g % 2) * QB:(g % 2 + 1) * QB],
                            start=(kt == 0), stop=(kt == NKT - 1),
                            tile_position=(0, g * 32),
                            skip_group_check=True,
                        )
                # ---- normalize ----
                for u, pair in ((uA, pA), (uB, pB)):
                    for sub in range(2):
                        g = (pair % 2) * 2 + sub
                        rcs = nrm_p.tile([1, QB], F32, tag="rcs", name="rcs")
                        nc.vector.tensor_copy(
                            rcs[:], seb[g * 32:g * 32 + 1, :])
                        rcr = nrm_p.tile([1, QB], F32, tag="rcr", name="rcr")
                        nc.vector.reciprocal_approx_fast(rcr[:], rcs[:])
                        bcf = nrm_p.tile([128, QB], F32, tag="bcf",
                                         name="bcf")
                        nc.gpsimd.partition_broadcast(bcf[:], rcr[:])
                        nc.vector.tensor_mul(
                            atts[qb][pair][sub * 64:(sub + 1) * 64, :],
                            u[sub * 64:(sub + 1) * 64, :],
                            bcf[sub * 64:(sub + 1) * 64, :])
        # tail: last q-block's output projection
        for eo in range(NET):
            outproj_group(3, eo)

    nc.compile()
    return nc


_CACHED = {}


def _get_program():
    if "nc" not in _CACHED:
        _CACHED["nc"] = build_program()
    return _CACHED["nc"]


def make_inputs(embeddings, wq, bq, wk, bk, wv, bv, wo, bo):
    """Host-side sharding: per-core input maps."""
    in_maps = []
    for c in range(N_CORES):
        b, half = c // 2, c % 2
        sl = slice(half * OL, (half + 1) * OL)
        in_maps.append({
            "xT": np.ascontiguousarray(embeddings[b].T).astype(NPBF16),
            "wqT": np.ascontiguousarray(wq[sl, :].T).astype(NPBF16),
            "wkT": np.ascontiguousarray(wk[sl, :].T).astype(NPBF16),
            "wvT": np.ascontiguousarray(wv[sl, :].T).astype(NPBF16),
            "woT": np.ascontiguousarray(wo[:, sl].T).astype(NPBF16),
            "bqc": np.ascontiguousarray(
                bq[sl].reshape(4, 128).T).astype(np.float32),
            "bkc": np.ascontiguousarray(
                bk[sl].reshape(4, 128).T).astype(np.float32),
            "bv": bv[sl].reshape(1, OL).astype(NPBF16),
        })
    return in_maps


def unshard(results, bo):
    out = np.empty((B, S, E), np.float32)
    for b in range(B):
        yt = results[2 * b]["yT"] + results[2 * b + 1]["yT"]
        out[b] = yt.T + bo[None, :]
    return out


def kernel(embeddings, wq, bq, wk, bk, wv, bv, wo, bo, _trace=False):
    embeddings = np.asarray(embeddings, np.float32)
    nc = _get_program()
    in_maps = make_inputs(
        embeddings, np.asarray(wq, np.float32), np.asarray(bq, np.float32),
        np.asarray(wk, np.float32), np.asarray(bk, np.float32),
        np.asarray(wv, np.float32), np.asarray(bv, np.float32),
        np.asarray(wo, np.float32), np.asarray(bo, np.float32))
    res = run_bass_kernel_spmd(
        nc, in_maps, core_ids=list(range(N_CORES)), trace=_trace)
    out = unshard(res.results, np.asarray(bo, np.float32))
    if _trace:
        kernel.last_result = res
    return out


# revision 14
# speedup vs baseline: 1.2733x; 1.2733x over previous
"""Multi-head attention kernel for 8 Trainium2 NeuronCores.

Problem: embeddings [4, 2048, 1024], 16 heads x 64 dim, torch nn.Linear
convention (x @ W.T + b) for Q/K/V/O projections.

Sharding: batch (4) x head-halves (2) -> 8 cores. Core c handles batch
c//2, local heads (c%2)*8..(c%2)*8+8. Output projection is row-sharded;
host sums the two partial outputs per batch element and adds bo.

Per-core dataflow (feature dims on partitions, ScalarE exp is the
critical engine and is kept fed by interleaving projection work into
the attention instruction stream):
  xT [1024e, 2048t] bf16 (host pre-transposed + cast)
  QT/KT [(h,d)=512, t] via PE, bias added on DVE during PSUM evac.
  V [t, (h,d)] via PE, bias via K=1 ones x bv matmul.
  Per head-quad (4 heads = 2 pairs), per q-block of 512, per k-tile:
    scores_T[k,q] row-paired matmuls (2 heads share the PE array),
    exp on ScalarE (1/8 scale folded in, no max subtraction needed),
    U[(2x64),q] col-paired matmuls, sumexp via 4 col-tiled M=1
    ones-matmuls into one PSUM bank (partitions 0/32/64/96).
  normalize: recip(sumexp) -> gpsimd partition-broadcast -> DVE mult.
  yT[e_out, t] = woT.T @ attn_T accumulated over 4 pair-tiles.
Host: out[b] = (yT[2b] + yT[2b+1]).T + bo.
"""

import sys

sys.path.insert(0, "/opt/trn_rl_repo")

import numpy as np
import ml_dtypes

import concourse.bass as bass
import concourse.bacc as bacc
import concourse.mybir as mybir
import concourse.tile as tile
from concourse.bass_utils import run_bass_kernel_spmd

BF16 = mybir.dt.bfloat16
F32 = mybir.dt.float32
NPBF16 = ml_dtypes.bfloat16

B, S, E = 4, 2048, 1024
H_LOC = 8          # local heads per core
D = 64             # head dim
OL = H_LOC * D     # 512 local output dim
N_CORES = 8
QB = 512           # query block (free dim of scores_T)
NQB = S // QB      # 4
NKT = S // 128     # 16 key tiles
NET = E // 128     # 8 embed tiles


def build_program():
    from contextlib import ExitStack

    nc = bacc.Bacc("TRN2", debug=False, num_devices=N_CORES)

    xT = nc.dram_tensor("xT", [E, S], BF16, kind="ExternalInput")
    wqT = nc.dram_tensor("wqT", [E, OL], BF16, kind="ExternalInput")
    wkT = nc.dram_tensor("wkT", [E, OL], BF16, kind="ExternalInput")
    wvT = nc.dram_tensor("wvT", [E, OL], BF16, kind="ExternalInput")
    woT = nc.dram_tensor("woT", [OL, E], BF16, kind="ExternalInput")
    bqc = nc.dram_tensor("bqc", [128, 4], F32, kind="ExternalInput")
    bkc = nc.dram_tensor("bkc", [128, 4], F32, kind="ExternalInput")
    bv = nc.dram_tensor("bv", [1, OL], BF16, kind="ExternalInput")
    yT = nc.dram_tensor("yT", [E, S], F32, kind="ExternalOutput")

    with tile.TileContext(nc) as tc, ExitStack() as est:
        xt_p = est.enter_context(tc.tile_pool(name="xt", bufs=NET))
        wq_p = est.enter_context(tc.tile_pool(name="wq", bufs=NET))
        wk_p = est.enter_context(tc.tile_pool(name="wk", bufs=NET))
        wv_p = est.enter_context(tc.tile_pool(name="wv", bufs=NET))
        wo_p = est.enter_context(tc.tile_pool(name="wo", bufs=4))
        bias_p = est.enter_context(tc.tile_pool(name="bias", bufs=4))
        qt_p = est.enter_context(tc.tile_pool(name="qt", bufs=4))
        kt_p = est.enter_context(tc.tile_pool(name="kt", bufs=4))
        vb_p = est.enter_context(tc.tile_pool(name="vb", bufs=NKT))
        pj_p = est.enter_context(tc.tile_pool(name="pj", bufs=1, space="PSUM"))
        sc_p = est.enter_context(tc.tile_pool(name="sc", bufs=2, space="PSUM"))
        u_p = est.enter_context(tc.tile_pool(name="u", bufs=2, space="PSUM"))
        se_p = est.enter_context(tc.tile_pool(name="se", bufs=1, space="PSUM"))
        ex_p = est.enter_context(tc.tile_pool(name="ex", bufs=20))
        at_p = est.enter_context(tc.tile_pool(name="at", bufs=16))
        nrm_p = est.enter_context(tc.tile_pool(name="nrm", bufs=2))
        ys_p = est.enter_context(tc.tile_pool(name="ys", bufs=2))

        # ---- load inputs ----
        xts = []
        for e in range(NET):
            t = xt_p.tile([128, S], BF16, tag="xt", name="xt")
            nc.sync.dma_start(t[:], xT[e * 128:(e + 1) * 128, :])
            xts.append(t)
        wts = {}
        for name, dram, pool in (
            ("q", wqT, wq_p), ("k", wkT, wk_p), ("v", wvT, wv_p),
        ):
            lst = []
            for e in range(NET):
                t = pool.tile([128, OL], BF16, tag="w" + name, name="w" + name)
                nc.sync.dma_start(t[:], dram[e * 128:(e + 1) * 128, :])
                lst.append(t)
            wts[name] = lst
        wos = []
        for p in range(4):
            t = wo_p.tile([128, E], BF16, tag="wo", name="wo")
            nc.sync.dma_start(t[:], woT[p * 128:(p + 1) * 128, :])
            wos.append(t)
        bqs = bias_p.tile([128, 4], F32, tag="bqc")
        bks = bias_p.tile([128, 4], F32, tag="bkc")
        bvs = bias_p.tile([1, OL], BF16, tag="bv")
        ones = bias_p.tile([1, 128], BF16, tag="ones")
        onecol = bias_p.tile([128, 1], BF16, tag="onecol")
        nc.sync.dma_start(bqs[:], bqc[:])
        nc.sync.dma_start(bks[:], bkc[:])
        nc.sync.dma_start(bvs[:], bv[:])
        nc.vector.memset(ones[:], 1.0)
        nc.vector.memset(onecol[:], 1.0)

        qts = [qt_p.tile([128, S], BF16, tag="qt", name="qt")
               for _ in range(4)]
        kts = [kt_p.tile([128, S], BF16, tag="kt", name="kt")
               for _ in range(4)]
        vbs = [vb_p.tile([128, OL], BF16, tag="vb", name="vb")
               for _ in range(NKT)]
        atts = [[at_p.tile([128, QB], BF16, tag="at", name="at")
                 for _ in range(4)] for _ in range(NQB)]

        # ---- projection / outproj group emitters (PE fillers) ----
        def qk_group(i, j, which):
            """Q or K projection for o-tile i, t-block j (one PSUM group)."""
            w = wts[which]
            bias_t = bqs if which == "q" else bks
            dest = qts[i] if which == "q" else kts[i]
            acc = pj_p.tile([128, QB], F32, tag="pj", name="pj")
            for e in range(NET):
                nc.tensor.matmul(
                    acc[:],
                    w[e][:, i * 128:(i + 1) * 128],
                    xts[e][:, j * QB:(j + 1) * QB],
                    start=(e == 0), stop=(e == NET - 1),
                )
            nc.vector.tensor_scalar_add(
                dest[:, j * QB:(j + 1) * QB], acc[:], bias_t[:, i:i + 1])

        def v_group(ti):
            acc = pj_p.tile([128, OL], F32, tag="pj", name="pjv")
            nc.tensor.matmul(
                acc[:], ones[0:1, :], bvs[0:1, :], start=True, stop=False)
            for e in range(NET):
                nc.tensor.matmul(
                    acc[:],
                    xts[e][:, ti * 128:(ti + 1) * 128],
                    wts["v"][e][:],
                    start=False, stop=(e == NET - 1),
                )
            nc.vector.tensor_copy(vbs[ti][:], acc[:])

        def outproj_group(qb, eo):
            y = pj_p.tile([128, QB], F32, tag="pj", name="y")
            for p2 in range(4):
                nc.tensor.matmul(
                    y[:],
                    wos[p2][:, eo * 128:(eo + 1) * 128],
                    atts[qb][p2][:],
                    start=(p2 == 0), stop=(p2 == 3),
                )
            ysb = ys_p.tile([128, QB], F32, tag="ys", name="ys")
            nc.vector.tensor_copy(ysb[:], y[:])
            nc.sync.dma_start(
                yT[eo * 128:(eo + 1) * 128, qb * QB:(qb + 1) * QB], ysb[:])

        # ---- filler schedule ----
        def qg(i, j):
            return lambda: qk_group(i, j, "q")

        def kg(i, j):
            return lambda: qk_group(i, j, "k")

        def vg(t):
            return lambda: v_group(t)

        def og(qb, e):
            return lambda: outproj_group(qb, e)

        # Flat software-pipelined step list: one step per (quad, qb,
        # pair-in-quad, kt). At step i the ScalarE exp for step i is
        # emitted first, then the scores matmuls for step i+1, then PE
        # filler groups, then the U / sumexp matmuls for step i (which
        # wait on exp i) -- so ScalarE always has its next input queued.
        step_list = []
        for quad in range(2):
            for qb in range(NQB):
                for pi in range(2):
                    for kt in range(NKT):
                        step_list.append((quad, qb, 2 * quad + pi, pi, kt))
        nsteps = len(step_list)

        def sidx(quad, qb, pi, kt):
            return ((quad * NQB + qb) * 2 + pi) * NKT + kt

        prologue = [qg(0, 0), kg(0, 0), vg(0), vg(1), vg(2)]
        sched = {}

        def put(step, th):
            sched.setdefault(step, []).append(th)

        for t in range(3, NKT):          # V(t) needed at step t
            put(t - 3, vg(t))
        for j in range(1, 4):            # K(0,j) read from step 4j-1
            put(4 * j - 4, kg(0, j))
        put(12, qg(1, 0))                # pair 1 starts at step 16
        put(13, kg(1, 0))
        for j in range(1, 4):            # K(1,j) read from step 16+4j-1
            put(16 + 4 * j - 4, kg(1, j))
        put(sidx(0, 0, 1, 12), qg(0, 1))   # Q(0,1) by step 32
        put(sidx(0, 1, 0, 12), qg(1, 1))   # Q(1,1) by step 48
        put(sidx(0, 1, 1, 4), qg(0, 2))    # Q(0,2) by step 64
        put(sidx(0, 1, 1, 12), qg(1, 2))
        put(sidx(0, 2, 0, 4), qg(0, 3))
        put(sidx(0, 2, 0, 12), qg(1, 3))
        # quad1 K/Q: needed from step 128 on
        put(sidx(0, 2, 1, 0), kg(2, 0))
        put(sidx(0, 2, 1, 4), kg(2, 1))
        put(sidx(0, 2, 1, 8), kg(2, 2))
        put(sidx(0, 2, 1, 12), kg(2, 3))
        put(sidx(0, 3, 0, 0), kg(3, 0))
        put(sidx(0, 3, 0, 4), kg(3, 1))
        put(sidx(0, 3, 0, 8), kg(3, 2))
        put(sidx(0, 3, 0, 12), kg(3, 3))
        put(sidx(0, 3, 1, 0), qg(2, 0))
        put(sidx(0, 3, 1, 8), qg(3, 0))
        put(sidx(1, 0, 0, 0), qg(2, 1))
        put(sidx(1, 0, 0, 8), qg(3, 1))
        put(sidx(1, 0, 1, 0), qg(2, 2))
        put(sidx(1, 0, 1, 8), qg(3, 2))
        put(sidx(1, 1, 0, 0), qg(2, 3))
        put(sidx(1, 1, 0, 8), qg(3, 3))
        for e in range(NET):             # outproj(qb) in (quad1, qb+1)
            put(sidx(1, 1, e % 2, 2 + 4 * (e // 2)), og(0, e))
            put(sidx(1, 2, e % 2, 2 + 4 * (e // 2)), og(1, e))
            put(sidx(1, 3, e % 2, 2 + 4 * (e // 2)), og(2, e))

        for th in prologue:
            th()

        # ---- attention ----
        def emit_scores(quad, qb, pair, kt):
            sc = sc_p.tile([128, 2 * QB], F32, tag="sc", name="sc")
            nc.tensor.matmul(
                sc[:, 0:QB],
                kts[pair][0:64, kt * 128:(kt + 1) * 128],
                qts[pair][0:64, qb * QB:(qb + 1) * QB],
                start=True, stop=True, tile_position=(0, 0),
            )
            nc.tensor.matmul(
                sc[:, QB:2 * QB],
                kts[pair][64:128, kt * 128:(kt + 1) * 128],
                qts[pair][64:128, qb * QB:(qb + 1) * QB],
                start=True, stop=True, tile_position=(64, 0),
            )
            return sc

        q0, q1, p1, _, k1 = step_list[0]
        pend_sc = emit_scores(q0, q1, p1, k1)
        cur = {}      # per-(quad,qb) state: uA, uB, seb, etA list
        for i, (quad, qb, pair, pi, kt) in enumerate(step_list):
            if (pi, kt) == (0, 0):
                cur["uA"] = u_p.tile([128, QB], F32, tag="u", name="uA")
                cur["uB"] = u_p.tile([128, QB], F32, tag="u", name="uB")
                cur["seb"] = se_p.tile([128, QB], F32, tag="se", name="seb")
                cur["etA"] = [None] * NKT
            # exp for this step
            et = ex_p.tile([128, 2 * QB], BF16, tag="ex", name="ex")
            nc.scalar.activation(
                et[:], pend_sc[:],
                mybir.ActivationFunctionType.Exp, scale=0.125)
            if pi == 0:
                cur["etA"][kt] = et
            # scores for next step
            if i + 1 < nsteps:
                nq, nqb, npair, _, nkt = step_list[i + 1]
                pend_sc = emit_scores(nq, nqb, npair, nkt)
            # fillers
            for th in sched.get(i, []):
                th()
            # U matmuls for this step
            u = cur["uA"] if pi == 0 else cur["uB"]
            for sub in range(2):
                hcol = (pair * 2 + sub) * D
                nc.tensor.matmul(
                    u[sub * 64:(sub + 1) * 64, :],
                    vbs[kt][:, hcol:hcol + D],
                    et[:, sub * QB:(sub + 1) * QB],
                    start=(kt == 0), stop=(kt == NKT - 1),
                    tile_position=(0, sub * 64),
                    skip_group_check=True,
                )
            if pi == 1:
                # quad-packed sumexp: 4 col-tiled M=1 matmuls, one bank
                seb = cur["seb"]
                epair = (cur["etA"][kt], et)
                for g in range(4):
                    nc.tensor.matmul(
                        seb[g * 32:g * 32 + 1, :],
                        onecol[:],
                        epair[g // 2][:, (g % 2) * QB:(g % 2 + 1) * QB],
                        start=(kt == 0), stop=(kt == NKT - 1),
                        tile_position=(0, g * 32),
                        skip_group_check=True,
                    )
                if kt == NKT - 1:
                    # ---- normalize the quad ----
                    pA, pB = 2 * quad, 2 * quad + 1
                    seb = cur["seb"]
                    for u2, pr in ((cur["uA"], pA), (cur["uB"], pB)):
                        for sub in range(2):
                            g = (pr % 2) * 2 + sub
                            rcs = nrm_p.tile([1, QB], F32, tag="rcs",
                                             name="rcs")
                            nc.vector.tensor_copy(
                                rcs[:], seb[g * 32:g * 32 + 1, :])
                            rcr = nrm_p.tile([1, QB], F32, tag="rcr",
                                             name="rcr")
                            nc.vector.reciprocal_approx_fast(rcr[:], rcs[:])
                            bcf = nrm_p.tile([128, QB], F32, tag="bcf",
                                             name="bcf")
                            nc.gpsimd.partition_broadcast(bcf[:], rcr[:])
                            nc.vector.tensor_mul(
                                atts[qb][pr][sub * 64:(sub + 1) * 64, :],
                                u2[sub * 64:(sub + 1) * 64, :],
                                bcf[sub * 64:(sub + 1) * 64, :])
        # tail: last q-block's output projection
        for eo in range(NET):
            outproj_group(3, eo)

    nc.compile()
    return nc


_CACHED = {}


def _get_program():
    if "nc" not in _CACHED:
        _CACHED["nc"] = build_program()
    return _CACHED["nc"]


def make_inputs(embeddings, wq, bq, wk, bk, wv, bv, wo, bo):
    """Host-side sharding: per-core input maps."""
    in_maps = []
    for c in range(N_CORES):
        b, half = c // 2, c % 2
        sl = slice(half * OL, (half + 1) * OL)
        in_maps.append({
            "xT": np.ascontiguousarray(embeddings[b].T).astype(NPBF16),
            "wqT": np.ascontiguousarray(wq[sl, :].T).astype(NPBF16),
            "wkT": np.ascontiguousarray(wk[sl, :].T).astype(NPBF16),
            "wvT": np.ascontiguousarray(wv[sl, :].T).astype(NPBF16),
            "woT": np.ascontiguousarray(wo[:, sl].T).astype(NPBF16),
            "bqc": np.ascontiguousarray(
                bq[sl].reshape(4, 128).T).astype(np.float32),
            "bkc": np.ascontiguousarray(
                bk[sl].reshape(4, 128).T).astype(np.float32),
            "bv": bv[sl].reshape(1, OL).astype(NPBF16),
        })
    return in_maps


def unshard(results, bo):
    out = np.empty((B, S, E), np.float32)
    for b in range(B):
        yt = results[2 * b]["yT"] + results[2 * b + 1]["yT"]
        out[b] = yt.T + bo[None, :]
    return out


def kernel(embeddings, wq, bq, wk, bk, wv, bv, wo, bo, _trace=False):
    embeddings = np.asarray(embeddings, np.float32)
    nc = _get_program()
    in_maps = make_inputs(
        embeddings, np.asarray(wq, np.float32), np.asarray(bq, np.float32),
        np.asarray(wk, np.float32), np.asarray(bk, np.float32),
        np.asarray(wv, np.float32), np.asarray(bv, np.float32),
        np.asarray(wo, np.float32), np.asarray(bo, np.float32))
    res = run_bass_kernel_spmd(
        nc, in_maps, core_ids=list(range(N_CORES)), trace=_trace)
    out = unshard(res.results, np.asarray(bo, np.float32))
    if _trace:
        kernel.last_result = res
    return out


# revision 15
# speedup vs baseline: 1.2902x; 1.0133x over previous
"""Multi-head attention kernel for 8 Trainium2 NeuronCores.

Problem: embeddings [4, 2048, 1024], 16 heads x 64 dim, torch nn.Linear
convention (x @ W.T + b) for Q/K/V/O projections.

Sharding: batch (4) x head-halves (2) -> 8 cores. Core c handles batch
c//2, local heads (c%2)*8..(c%2)*8+8. Output projection is row-sharded;
host sums the two partial outputs per batch element and adds bo.

Per-core dataflow (feature dims on partitions, ScalarE exp is the
critical engine and is kept fed by interleaving projection work into
the attention instruction stream):
  xT [1024e, 2048t] bf16 (host pre-transposed + cast)
  QT/KT [(h,d)=512, t] via PE, bias added on DVE during PSUM evac.
  V [t, (h,d)] via PE, bias via K=1 ones x bv matmul.
  Per head-quad (4 heads = 2 pairs), per q-block of 512, per k-tile:
    scores_T[k,q] row-paired matmuls (2 heads share the PE array),
    exp on ScalarE (1/8 scale folded in, no max subtraction needed),
    U[(2x64),q] col-paired matmuls, sumexp via 4 col-tiled M=1
    ones-matmuls into one PSUM bank (partitions 0/32/64/96).
  normalize: recip(sumexp) -> gpsimd partition-broadcast -> DVE mult.
  yT[e_out, t] = woT.T @ attn_T accumulated over 4 pair-tiles.
Host: out[b] = (yT[2b] + yT[2b+1]).T + bo.
"""

import sys

sys.path.insert(0, "/opt/trn_rl_repo")

import numpy as np
import ml_dtypes

import concourse.bass as bass
import concourse.bacc as bacc
import concourse.mybir as mybir
import concourse.tile as tile
from concourse.bass_utils import run_bass_kernel_spmd

BF16 = mybir.dt.bfloat16
F32 = mybir.dt.float32
NPBF16 = ml_dtypes.bfloat16

B, S, E = 4, 2048, 1024
H_LOC = 8          # local heads per core
D = 64             # head dim
OL = H_LOC * D     # 512 local output dim
N_CORES = 8
QB = 512           # query block (free dim of scores_T)
NQB = S // QB      # 4
NKT = S // 128     # 16 key tiles
NET = E // 128     # 8 embed tiles


def build_program():
    from contextlib import ExitStack

    nc = bacc.Bacc("TRN2", debug=False, num_devices=N_CORES)

    xT = nc.dram_tensor("xT", [E, S], BF16, kind="ExternalInput")
    wqT = nc.dram_tensor("wqT", [E, OL], BF16, kind="ExternalInput")
    wkT = nc.dram_tensor("wkT", [E, OL], BF16, kind="ExternalInput")
    wvT = nc.dram_tensor("wvT", [E, OL], BF16, kind="ExternalInput")
    woT = nc.dram_tensor("woT", [OL, E], BF16, kind="ExternalInput")
    bqc = nc.dram_tensor("bqc", [128, 4], F32, kind="ExternalInput")
    bkc = nc.dram_tensor("bkc", [128, 4], F32, kind="ExternalInput")
    bv = nc.dram_tensor("bv", [1, OL], BF16, kind="ExternalInput")
    yT = nc.dram_tensor("yT", [E, S], F32, kind="ExternalOutput")

    with tile.TileContext(nc) as tc, ExitStack() as est:
        xt_p = est.enter_context(tc.tile_pool(name="xt", bufs=NET))
        wq_p = est.enter_context(tc.tile_pool(name="wq", bufs=NET))
        wk_p = est.enter_context(tc.tile_pool(name="wk", bufs=NET))
        wv_p = est.enter_context(tc.tile_pool(name="wv", bufs=NET))
        wo_p = est.enter_context(tc.tile_pool(name="wo", bufs=4))
        bias_p = est.enter_context(tc.tile_pool(name="bias", bufs=4))
        qt_p = est.enter_context(tc.tile_pool(name="qt", bufs=4))
        kt_p = est.enter_context(tc.tile_pool(name="kt", bufs=4))
        vb_p = est.enter_context(tc.tile_pool(name="vb", bufs=NKT))
        pj_p = est.enter_context(tc.tile_pool(name="pj", bufs=1, space="PSUM"))
        sc_p = est.enter_context(tc.tile_pool(name="sc", bufs=2, space="PSUM"))
        u_p = est.enter_context(tc.tile_pool(name="u", bufs=2, space="PSUM"))
        se_p = est.enter_context(tc.tile_pool(name="se", bufs=1, space="PSUM"))
        ex_p = est.enter_context(tc.tile_pool(name="ex", bufs=20))
        at_p = est.enter_context(tc.tile_pool(name="at", bufs=16))
        nrm_p = est.enter_context(tc.tile_pool(name="nrm", bufs=2))
        ys_p = est.enter_context(tc.tile_pool(name="ys", bufs=2))

        # ---- load inputs ----
        xts = []
        for e in range(NET):
            t = xt_p.tile([128, S], BF16, tag="xt", name="xt")
            nc.sync.dma_start(t[:], xT[e * 128:(e + 1) * 128, :])
            xts.append(t)
        wts = {}
        for name, dram, pool in (
            ("q", wqT, wq_p), ("k", wkT, wk_p), ("v", wvT, wv_p),
        ):
            lst = []
            for e in range(NET):
                t = pool.tile([128, OL], BF16, tag="w" + name, name="w" + name)
                nc.sync.dma_start(t[:], dram[e * 128:(e + 1) * 128, :])
                lst.append(t)
            wts[name] = lst
        wos = [wo_p.tile([128, E], BF16, tag="wo", name="wo")
               for _ in range(4)]
        bqs = bias_p.tile([128, 4], F32, tag="bqc")
        bks = bias_p.tile([128, 4], F32, tag="bkc")
        bvs = bias_p.tile([1, OL], BF16, tag="bv")
        ones = bias_p.tile([1, 128], BF16, tag="ones")
        onecol = bias_p.tile([128, 1], BF16, tag="onecol")
        nc.sync.dma_start(bqs[:], bqc[:])
        nc.sync.dma_start(bks[:], bkc[:])
        nc.sync.dma_start(bvs[:], bv[:])
        nc.vector.memset(ones[:], 1.0)
        nc.vector.memset(onecol[:], 1.0)

        qts = [qt_p.tile([128, S], BF16, tag="qt", name="qt")
               for _ in range(4)]
        kts = [kt_p.tile([128, S], BF16, tag="kt", name="kt")
               for _ in range(4)]
        vbs = [vb_p.tile([128, OL], BF16, tag="vb", name="vb")
               for _ in range(NKT)]
        atts = [[at_p.tile([128, QB], BF16, tag="at", name="at")
                 for _ in range(4)] for _ in range(NQB)]

        # ---- projection / outproj group emitters (PE fillers) ----
        def qk_group(i, j, which):
            """Q or K projection for o-tile i, t-block j (one PSUM group)."""
            w = wts[which]
            bias_t = bqs if which == "q" else bks
            dest = qts[i] if which == "q" else kts[i]
            acc = pj_p.tile([128, QB], F32, tag="pj", name="pj")
            for e in range(NET):
                nc.tensor.matmul(
                    acc[:],
                    w[e][:, i * 128:(i + 1) * 128],
                    xts[e][:, j * QB:(j + 1) * QB],
                    start=(e == 0), stop=(e == NET - 1),
                )
            nc.vector.tensor_scalar_add(
                dest[:, j * QB:(j + 1) * QB], acc[:], bias_t[:, i:i + 1])

        def v_group(ti):
            acc = pj_p.tile([128, OL], F32, tag="pj", name="pjv")
            nc.tensor.matmul(
                acc[:], ones[0:1, :], bvs[0:1, :], start=True, stop=False)
            for e in range(NET):
                nc.tensor.matmul(
                    acc[:],
                    xts[e][:, ti * 128:(ti + 1) * 128],
                    wts["v"][e][:],
                    start=False, stop=(e == NET - 1),
                )
            nc.vector.tensor_copy(vbs[ti][:], acc[:])

        def outproj_group(qb, eo, pool=None, tag="pj"):
            y = (pool or pj_p).tile([128, QB], F32, tag=tag, name="y")
            for p2 in range(4):
                nc.tensor.matmul(
                    y[:],
                    wos[p2][:, eo * 128:(eo + 1) * 128],
                    atts[qb][p2][:],
                    start=(p2 == 0), stop=(p2 == 3),
                )
            ysb = ys_p.tile([128, QB], F32, tag="ys", name="ys")
            nc.vector.tensor_copy(ysb[:], y[:])
            nc.sync.dma_start(
                yT[eo * 128:(eo + 1) * 128, qb * QB:(qb + 1) * QB], ysb[:])

        # ---- filler schedule ----
        def qg(i, j):
            return lambda: qk_group(i, j, "q")

        def kg(i, j):
            return lambda: qk_group(i, j, "k")

        def vg(t):
            return lambda: v_group(t)

        def og(qb, e):
            return lambda: outproj_group(qb, e)

        # Flat software-pipelined step list: one step per (quad, qb,
        # pair-in-quad, kt). At step i the ScalarE exp for step i is
        # emitted first, then the scores matmuls for step i+1, then PE
        # filler groups, then the U / sumexp matmuls for step i (which
        # wait on exp i) -- so ScalarE always has its next input queued.
        step_list = []
        for quad in range(2):
            for qb in range(NQB):
                for pi in range(2):
                    for kt in range(NKT):
                        step_list.append((quad, qb, 2 * quad + pi, pi, kt))
        nsteps = len(step_list)

        def sidx(quad, qb, pi, kt):
            return ((quad * NQB + qb) * 2 + pi) * NKT + kt

        prologue = [qg(0, 0), kg(0, 0), vg(0), vg(1), vg(2)]
        sched = {}

        def put(step, th):
            sched.setdefault(step, []).append(th)

        for t in range(3, NKT):          # V(t) needed at step t
            put(t - 3, vg(t))
        for j in range(1, 4):            # K(0,j) read from step 4j-1
            put(4 * j - 4, kg(0, j))
        put(12, qg(1, 0))                # pair 1 starts at step 16
        put(13, kg(1, 0))
        for j in range(1, 4):            # K(1,j) read from step 16+4j-1
            put(16 + 4 * j - 4, kg(1, j))
        put(sidx(0, 0, 1, 12), qg(0, 1))   # Q(0,1) by step 32
        put(sidx(0, 1, 0, 12), qg(1, 1))   # Q(1,1) by step 48
        put(sidx(0, 1, 1, 4), qg(0, 2))    # Q(0,2) by step 64
        put(sidx(0, 1, 1, 12), qg(1, 2))
        put(sidx(0, 2, 0, 4), qg(0, 3))
        put(sidx(0, 2, 0, 12), qg(1, 3))
        # quad1 K/Q: needed from step 128 on
        put(sidx(0, 2, 1, 0), kg(2, 0))
        put(sidx(0, 2, 1, 4), kg(2, 1))
        put(sidx(0, 2, 1, 8), kg(2, 2))
        put(sidx(0, 2, 1, 12), kg(2, 3))
        put(sidx(0, 3, 0, 0), kg(3, 0))
        put(sidx(0, 3, 0, 4), kg(3, 1))
        put(sidx(0, 3, 0, 8), kg(3, 2))
        put(sidx(0, 3, 0, 12), kg(3, 3))
        put(sidx(0, 3, 1, 0), qg(2, 0))
        put(sidx(0, 3, 1, 8), qg(3, 0))
        put(sidx(1, 0, 0, 0), qg(2, 1))
        put(sidx(1, 0, 0, 8), qg(3, 1))
        put(sidx(1, 0, 1, 0), qg(2, 2))
        put(sidx(1, 0, 1, 8), qg(3, 2))
        put(sidx(1, 1, 0, 0), qg(2, 3))
        put(sidx(1, 1, 0, 8), qg(3, 3))
        for e in range(NET):             # outproj(qb) in (quad1, qb+1)
            put(sidx(1, 1, e % 2, 2 + 4 * (e // 2)), og(0, e))
            put(sidx(1, 2, e % 2, 2 + 4 * (e // 2)), og(1, e))
            put(sidx(1, 3, e % 2, 2 + 4 * (e // 2)), og(2, e))

        for th in prologue:
            th()
        for p in range(4):
            nc.sync.dma_start(wos[p][:], woT[p * 128:(p + 1) * 128, :])

        # ---- attention ----
        def emit_scores(quad, qb, pair, kt):
            sc = sc_p.tile([128, 2 * QB], F32, tag="sc", name="sc")
            nc.tensor.matmul(
                sc[:, 0:QB],
                kts[pair][0:64, kt * 128:(kt + 1) * 128],
                qts[pair][0:64, qb * QB:(qb + 1) * QB],
                start=True, stop=True, tile_position=(0, 0),
            )
            nc.tensor.matmul(
                sc[:, QB:2 * QB],
                kts[pair][64:128, kt * 128:(kt + 1) * 128],
                qts[pair][64:128, qb * QB:(qb + 1) * QB],
                start=True, stop=True, tile_position=(64, 0),
            )
            return sc

        q0, q1, p1, _, k1 = step_list[0]
        pend_sc = emit_scores(q0, q1, p1, k1)
        cur = {}      # per-(quad,qb) state: uA, uB, seb, etA list
        for i, (quad, qb, pair, pi, kt) in enumerate(step_list):
            if (pi, kt) == (0, 0):
                cur["uA"] = u_p.tile([128, QB], F32, tag="u", name="uA")
                cur["uB"] = u_p.tile([128, QB], F32, tag="u", name="uB")
                cur["seb"] = se_p.tile([128, QB], F32, tag="se", name="seb")
                cur["etA"] = [None] * NKT
            # exp for this step
            et = ex_p.tile([128, 2 * QB], BF16, tag="ex", name="ex")
            nc.scalar.activation(
                et[:], pend_sc[:],
                mybir.ActivationFunctionType.Exp, scale=0.125)
            if pi == 0:
                cur["etA"][kt] = et
            # scores for next step
            if i + 1 < nsteps:
                nq, nqb, npair, _, nkt = step_list[i + 1]
                pend_sc = emit_scores(nq, nqb, npair, nkt)
            # fillers
            for th in sched.get(i, []):
                th()
            # U matmuls for this step
            u = cur["uA"] if pi == 0 else cur["uB"]
            for sub in range(2):
                hcol = (pair * 2 + sub) * D
                nc.tensor.matmul(
                    u[sub * 64:(sub + 1) * 64, :],
                    vbs[kt][:, hcol:hcol + D],
                    et[:, sub * QB:(sub + 1) * QB],
                    start=(kt == 0), stop=(kt == NKT - 1),
                    tile_position=(0, sub * 64),
                    skip_group_check=True,
                )
            if pi == 1:
                # quad-packed sumexp: 4 col-tiled M=1 matmuls, one bank
                seb = cur["seb"]
                epair = (cur["etA"][kt], et)
                for g in range(4):
                    nc.tensor.matmul(
                        seb[g * 32:g * 32 + 1, :],
                        onecol[:],
                        epair[g // 2][:, (g % 2) * QB:(g % 2 + 1) * QB],
                        start=(kt == 0), stop=(kt == NKT - 1),
                        tile_position=(0, g * 32),
                        skip_group_check=True,
                    )
                if kt == NKT - 1:
                    # ---- normalize the quad ----
                    pA, pB = 2 * quad, 2 * quad + 1
                    seb = cur["seb"]
                    for u2, pr in ((cur["uA"], pA), (cur["uB"], pB)):
                        for sub in range(2):
                            g = (pr % 2) * 2 + sub
                            rcs = nrm_p.tile([1, QB], F32, tag="rcs",
                                             name="rcs")
                            nc.vector.tensor_copy(
                                rcs[:], seb[g * 32:g * 32 + 1, :])
                            rcr = nrm_p.tile([1, QB], F32, tag="rcr",
                                             name="rcr")
                            nc.vector.reciprocal_approx_fast(rcr[:], rcs[:])
                            bcf = nrm_p.tile([128, QB], F32, tag="bcf",
                                             name="bcf")
                            nc.gpsimd.partition_broadcast(bcf[:], rcr[:])
                            nc.vector.tensor_mul(
                                atts[qb][pr][sub * 64:(sub + 1) * 64, :],
                                u2[sub * 64:(sub + 1) * 64, :],
                                bcf[sub * 64:(sub + 1) * 64, :])
        # tail: last q-block's output projection (scores pool is free
        # by now -- use its banks so the groups pipeline)
        for eo in range(NET):
            outproj_group(3, eo, pool=sc_p, tag="sc")

    nc.compile()
    return nc


_CACHED = {}


def _get_program():
    if "nc" not in _CACHED:
        _CACHED["nc"] = build_program()
    return _CACHED["nc"]


def make_inputs(embeddings, wq, bq, wk, bk, wv, bv, wo, bo):
    """Host-side sharding: per-core input maps."""
    in_maps = []
    for c in range(N_CORES):
        b, half = c // 2, c % 2
        sl = slice(half * OL, (half + 1) * OL)
        in_maps.append({
            "xT": np.ascontiguousarray(embeddings[b].T).astype(NPBF16),
            "wqT": np.ascontiguousarray(wq[sl, :].T).astype(NPBF16),
            "wkT": np.ascontiguousarray(wk[sl, :].T).astype(NPBF16),
            "wvT": np.ascontiguousarray(wv[sl, :].T).astype(NPBF16),
            "woT": np.ascontiguousarray(wo[:, sl].T).astype(NPBF16),
            "bqc": np.ascontiguousarray(
                bq[sl].reshape(4, 128).T).astype(np.float32),
            "bkc": np.ascontiguousarray(
                bk[sl].reshape(4, 128).T).astype(np.float32),
            "bv": bv[sl].reshape(1, OL).astype(NPBF16),
        })
    return in_maps


def unshard(results, bo):
    out = np.empty((B, S, E), np.float32)
    for b in range(B):
        yt = results[2 * b]["yT"] + results[2 * b + 1]["yT"]
        out[b] = yt.T + bo[None, :]
    return out


def kernel(embeddings, wq, bq, wk, bk, wv, bv, wo, bo, _trace=False):
    embeddings = np.asarray(embeddings, np.float32)
    nc = _get_program()
    in_maps = make_inputs(
        embeddings, np.asarray(wq, np.float32), np.asarray(bq, np.float32),
        np.asarray(wk, np.float32), np.asarray(bk, np.float32),
        np.asarray(wv, np.float32), np.asarray(bv, np.float32),
        np.asarray(wo, np.float32), np.asarray(bo, np.float32))
    res = run_bass_kernel_spmd(
        nc, in_maps, core_ids=list(range(N_CORES)), trace=_trace)
    out = unshard(res.results, np.asarray(bo, np.float32))
    if _trace:
        kernel.last_result = res
    return out
